# revision 1
# baseline (speedup 1.0000x reference)
"""Trainium2 Bass kernel for MultiHeadSyntonicAttention.

Problem: B=2, S=2048, D=1024, H=16 heads, DH=64.
  q/k/v = Linear(query/key/value); per-head gnosis gate
  gn = sigmoid(k . wg + bg); scores = (q k^T / sqrt(dh)) * (1+gn);
  out = softmax(scores) v;  out = ((out Wo+bo) Wd+bd) Wh+bh.

Sharding (8 cores): core c -> batch b=c//4, head-group g=c%4 (4 heads).
Each core computes its heads' attention and a row-slice partial of the
fused output projection Wf = Wo@Wd@Wh; host sums 4 partials per batch.

Device layout (everything "transposed", tokens on the free axis):
  host ships qT/kT/vT = x[b].T  [D=1024, S=2048] bf16
  QT[c,s] (c = head-local dim, 256 rows) = Wq_s^T qT     (lhsT=Wq_s nat.)
  K gating folded into K: K' = KT * (1+gn[head, s])
  ST[k,q] = K'T^T Q                      (contract dh=64)
  PT = exp(ST/8)  bf16                   (max-sub skipped; scores bounded)
  OT[0:64,q] accum over k-blocks: lhsT=[V_head | ones] -> row 64 = denom
  ctxT = OT[0:64]/OT[64]  -> partial = ctxT^T Wf_s  [2048, 1024] f32

PSUM is 8 banks x 512 f32; attention runs in q-halves of 1024 so score
tiles (2 banks) and PV accumulators (2 banks) double-buffer within 8.
"""

import sys

sys.path.insert(0, "/opt/trn_rl_repo")

import numpy as np
import ml_dtypes

BF16 = ml_dtypes.bfloat16

B, S, D, H = 2, 2048, 1024, 16
DH = D // H          # 64
HPC = 4              # heads per core
C = HPC * DH         # 256 head-local columns per core
NCORES = 8
ND = D // 128        # 8 d-chunks
NSB = S // 128       # 16 s-blocks
QW = 1024            # attention q-tile width
NQH = S // QW        # 2 q-halves

_nc_cache = {}


def build_bass():
    import concourse.bass as bass
    import concourse.mybir as mybir
    import concourse.tile as tile
    from concourse import bacc

    f32 = mybir.dt.float32
    bf16 = mybir.dt.bfloat16
    Alu = mybir.AluOpType
    Act = mybir.ActivationFunctionType

    nc = bacc.Bacc(None, target_bir_lowering=False, name="syntonic_attn")

    qT_d = nc.dram_tensor("qT", [D, S], bf16, kind="ExternalInput")
    kT_d = nc.dram_tensor("kT", [D, S], bf16, kind="ExternalInput")
    vT_d = nc.dram_tensor("vT", [D, S], bf16, kind="ExternalInput")
    wq_d = nc.dram_tensor("wq", [D, C], bf16, kind="ExternalInput")
    wk_d = nc.dram_tensor("wk", [D, C], bf16, kind="ExternalInput")
    wv_d = nc.dram_tensor("wv", [D, C], bf16, kind="ExternalInput")
    wf_d = nc.dram_tensor("wf", [C, D], bf16, kind="ExternalInput")
    wg4_d = nc.dram_tensor("wg4", [C, HPC], bf16, kind="ExternalInput")
    bq_d = nc.dram_tensor("bq", [1, C], bf16, kind="ExternalInput")
    bk_d = nc.dram_tensor("bk", [1, C], bf16, kind="ExternalInput")
    bv_d = nc.dram_tensor("bv", [1, C], bf16, kind="ExternalInput")
    bg_d = nc.dram_tensor("bg128", [128, 1], f32, kind="ExternalInput")
    out_d = nc.dram_tensor("out", [S, D], bf16, kind="ExternalOutput")

    AW = 1024            # attention q-tile width (2 PSUM banks)
    NA = S // AW         # 2

    with tile.TileContext(nc) as tc:
        with (
            tc.tile_pool(name="res", bufs=1) as res,
            tc.tile_pool(name="acts", bufs=1) as acts,
            tc.tile_pool(name="work", bufs=2) as work,
            tc.tile_pool(name="outp", bufs=3) as outp,
            tc.tile_pool(name="psum", bufs=1, space="PSUM") as psum,
        ):
            # ---------------- resident input tiles ----------------
            qT = [res.tile([128, S], bf16, tag=f"qT{i}", name=f"qT{i}") for i in range(ND)]
            kT = [res.tile([128, S], bf16, tag=f"kT{i}", name=f"kT{i}") for i in range(ND)]
            vT = [res.tile([128, S], bf16, tag=f"vT{i}", name=f"vT{i}") for i in range(ND)]
            wq = [res.tile([128, C], bf16, tag=f"wq{i}", name=f"wq{i}") for i in range(ND)]
            wk = [res.tile([128, C], bf16, tag=f"wk{i}", name=f"wk{i}") for i in range(ND)]
            wv = [res.tile([128, C], bf16, tag=f"wv{i}", name=f"wv{i}") for i in range(ND)]
            wf = [res.tile([128, D], bf16, tag=f"wf{i}", name=f"wf{i}") for i in range(2)]
            wg4 = [res.tile([128, HPC], bf16, tag=f"wg4{i}", name=f"wg4{i}") for i in range(2)]
            wg4c = [res.tile([128, HPC], bf16, tag=f"wg4c{i}", name=f"wg4c{i}")
                    for i in range(2)]
            bq = res.tile([1, C], bf16, tag="bq")
            bk = res.tile([1, C], bf16, tag="bk")
            bv = res.tile([1, C], bf16, tag="bv")
            bg128 = res.tile([128, 1], f32, tag="bg128")
            ones = res.tile([1, 512], bf16, tag="ones")
            c8 = res.tile([128, 1], f32, tag="c8")

            # K path first (gates need all of K), then Q, then V
            for i in range(ND):
                nc.sync.dma_start(wk[i][:], wk_d[i * 128:(i + 1) * 128, :])
                nc.sync.dma_start(kT[i][:], kT_d[i * 128:(i + 1) * 128, :])
            for i in range(2):
                nc.sync.dma_start(wg4[i][:], wg4_d[i * 128:(i + 1) * 128, :])
                nc.vector.tensor_copy(wg4c[i][:], wg4[i][:])
            nc.sync.dma_start(bk[:], bk_d[:])
            nc.sync.dma_start(bg128[:], bg_d[:])
            for i in range(ND):
                nc.sync.dma_start(wq[i][:], wq_d[i * 128:(i + 1) * 128, :])
                nc.sync.dma_start(qT[i][:], qT_d[i * 128:(i + 1) * 128, :])
            nc.sync.dma_start(bq[:], bq_d[:])
            for i in range(ND):
                nc.sync.dma_start(wv[i][:], wv_d[i * 128:(i + 1) * 128, :])
            nc.sync.dma_start(bv[:], bv_d[:])
            for sh in range(2):
                cols = slice(sh * 1024, (sh + 1) * 1024)
                for i in range(ND):
                    nc.sync.dma_start(vT[i][:, cols], vT_d[i * 128:(i + 1) * 128, cols])
            for i in range(2):
                nc.sync.dma_start(wf[i][:], wf_d[i * 128:(i + 1) * 128, :])
            nc.vector.memset(ones[:], 1.0)
            nc.vector.memset(c8[:], 0.125)

            QT = [acts.tile([128, S], bf16, tag=f"QT{i}", name=f"QT{i}") for i in range(2)]
            KT = [acts.tile([128, S], bf16, tag=f"KT{i}", name=f"KT{i}") for i in range(2)]
            ctxT = [acts.tile([128, S], bf16, tag=f"ctxT{i}", name=f"ctxT{i}") for i in range(2)]
            gscT = [acts.tile([128, HPC], f32, tag=f"gsc{i}", name=f"gsc{i}")
                    for i in range(NSB)]
            V = [acts.tile([128, HPC * 2 * DH], bf16, tag=f"V{i}", name=f"V{i}")
                 for i in range(NSB)]

            pj = [0]

            def project_T(w_tiles, x_tiles, bias, dest, cb, a, pname, tag=None, w=None):
                # one 512-wide output chunk: dest[:, a*512...] (+ rank-1 bias)
                w = w if w else 512
                qs = slice(a * w, (a + 1) * w)
                tg = tag if tag else f"st{pj[0] % 2}"
                ps = psum.tile([128, 512], f32, tag=tg, name=pname)
                pj[0] += 1
                for dc in range(ND):
                    nc.tensor.matmul(
                        ps[:, 0:w],
                        w_tiles[dc][:, cb * 128:(cb + 1) * 128],
                        x_tiles[dc][:, qs],
                        start=(dc == 0),
                        stop=False,
                    )
                nc.tensor.matmul(
                    ps[:, 0:w], bias[0:1, cb * 128:(cb + 1) * 128], ones[0:1, 0:w],
                    start=False, stop=True,
                )
                nc.vector.tensor_copy(dest[:, qs], ps[:, 0:w])

            def emit_gates(sb):
                gps = psum.tile([128, HPC], f32, tag="bg", name=f"gps{sb}")
                for cc in range(2):
                    nc.tensor.matmul(
                        gps[:],
                        KT[cc][:, sb * 128:(sb + 1) * 128],
                        wg4c[cc][:],
                        start=(cc == 0),
                        stop=(cc == 1),
                    )
                gn = work.tile([128, HPC], f32, tag="gn", name=f"gn{sb}", bufs=2)
                nc.scalar.activation(gn[:], gps[:], Act.Sigmoid, bias=bg128[:], scale=1.0)
                nc.scalar.activation(gscT[sb][:], gn[:], Act.Identity,
                                     bias=c8[:], scale=c8[:])

            def emit_vproj(sb):
                nc.vector.memset(V[sb][:], 1.0)
                ps = psum.tile([128, C], f32, tag="bg", name=f"psv{sb}")
                for dc in range(ND):
                    nc.tensor.matmul(
                        ps[:],
                        vT[dc][:, sb * 128:(sb + 1) * 128],
                        wv[dc][:],
                        start=(dc == 0),
                        stop=False,
                    )
                nc.tensor.matmul(ps[:], ones[0:1, 0:128], bv[:], start=False, stop=True)
                nc.vector.tensor_copy(
                    V[sb][:].rearrange("p (h x) -> p h x", h=HPC)[:, :, 0:DH],
                    ps[:].rearrange("p (h x) -> p h x", h=HPC),
                )

            # ------------- foreground pre-phase: K, gates, Q[cb0], V[0:4] -------------
            for cb in range(2):
                for a in range(4):
                    project_T(wk, kT, bk, KT[cb], cb, a, f"psk{cb}{a}")
            for sb in range(NSB):
                emit_gates(sb)
            for a in range(4):
                project_T(wq, qT, bq, QT[0], 0, a, f"psq0{a}")
            # V[0..7] need only the first vT column-half; their projection
            # overlaps the (DMA-bound) pre-phase where PE is otherwise idle
            for sb in range(8):
                emit_vproj(sb)

            # background emitters drained inside pair-0's attention loop
            # (V[sb] deadline: PV(sb) at iteration sb+1; every-2nd pop from
            # it=1 gives V[sb] at it=2(sb-8)+1 <= sb+1 for sb >= 8)
            background = [(lambda sb=sb: emit_vproj(sb)) for sb in range(8, NSB)]
            background += [
                (lambda a=a: project_T(wq, qT, bq, QT[1], 1, a, f"psq1{a}", tag="bg"))
                for a in range(4)
            ]

            # ---------------- attention (head-serial, AW=1024) ----------------
            # Software pipeline: PV lags QK by one kb so no PE instruction ever
            # waits on an in-flight exp; dummy matmuls keep the PE HAM-warm.
            warm = psum.tile([128, 128], f32, tag="warm", name="warm")

            def dummy_mm():
                nc.tensor.matmul(warm[:], wf[0][:, 0:128], wf[1][:, 0:128],
                                 start=True, stop=True, skip_group_check=True)

            it = [0]
            for cb in range(2):
                for hh in range(2):
                    h = 2 * cb + hh
                    po = hh * 64
                    vsl = slice(h * 2 * DH, (h + 1) * 2 * DH)
                    for a in range(NA):
                        qs = slice(a * AW, (a + 1) * AW)
                        ot = psum.tile([128, AW], f32, tag="ot", name=f"ot{h}{a}")
                        pts = {}
                        for kb in range(NSB):
                            it[0] += 1
                            if background and it[0] % 2 == 1:
                                background.pop(0)()
                            kslc = slice(kb * 128, (kb + 1) * 128)
                            st = psum.tile([128, AW], f32, tag=f"st{kb % 2}",
                                           name=f"st{h}{a}{kb}")
                            for qc in range(AW // 512):
                                cs = slice(qc * 512, (qc + 1) * 512)
                                aqs = slice(a * AW + qc * 512, a * AW + (qc + 1) * 512)
                                nc.tensor.matmul(st[:, cs], KT[cb][po:po + 64, kslc],
                                                 QT[cb][po:po + 64, aqs],
                                                 start=True, stop=True)
                            pt = work.tile([128, AW], bf16, tag="pt",
                                           name=f"pt{h}{a}{kb}", bufs=3)
                            nc.scalar.activation(pt[:], st[:], Act.Exp,
                                                 scale=gscT[kb][:, h:h + 1])
                            pts[kb] = pt
                            dummy_mm()
                            dummy_mm()
                            if kb > 0:
                                ptp = pts.pop(kb - 1)
                                for qc in range(AW // 512):
                                    cs = slice(qc * 512, (qc + 1) * 512)
                                    nc.tensor.matmul(
                                        ot[:, cs], V[kb - 1][:, vsl],
                                        ptp[:, cs],
                                        start=(kb == 1), stop=False,
                                        skip_group_check=True)
                        ptp = pts.pop(NSB - 1)
                        for qc in range(AW // 512):
                            cs = slice(qc * 512, (qc + 1) * 512)
                            nc.tensor.matmul(
                                ot[:, cs], V[NSB - 1][:, vsl],
                                ptp[:, cs],
                                start=False, stop=True, skip_group_check=True)
                        # fast psum release, then normalize on SBUF
                        otc = work.tile([128, AW], f32, tag="otc",
                                        name=f"otc{h}{a}", bufs=2)
                        nc.vector.tensor_copy(otc[:], ot[:])
                        rec = work.tile([DH, AW], f32, tag="rec",
                                        name=f"rec{h}{a}", bufs=2)
                        nc.vector.reciprocal(rec[:], otc[DH:2 * DH, :])
                        nc.vector.tensor_tensor(
                            ctxT[cb][po:po + 64, qs],
                            otc[0:DH, :],
                            rec[:],
                            Alu.mult,
                        )

            # ---------------- fused output projection ----------------
            for qb in range(NSB):
                tg = ("st0", "st1") if qb % 2 == 0 else ("ot", "bg")
                ps0 = psum.tile([128, 512], f32, tag=tg[0], name=f"pso{qb}a")
                ps1 = psum.tile([128, 512], f32, tag=tg[1], name=f"pso{qb}b")
                for cc in range(2):
                    lhsT = ctxT[cc][:, qb * 128:(qb + 1) * 128]
                    for oc, ps in enumerate((ps0, ps1)):
                        nc.tensor.matmul(
                            ps[:],
                            lhsT,
                            wf[cc][:, oc * 512:(oc + 1) * 512],
                            start=(cc == 0),
                            stop=(cc == 1),
                        )
                ob = outp.tile([128, D], bf16, tag="ob", name=f"ob{qb}")
                nc.vector.tensor_copy(ob[:, 0:512], ps0[:])
                nc.vector.tensor_copy(ob[:, 512:1024], ps1[:])
                nc.sync.dma_start(out_d[qb * 128:(qb + 1) * 128, :], ob[:])

    nc.finalize()
    return nc


def get_nc():
    if "nc" not in _nc_cache:
        _nc_cache["nc"] = build_bass()
    return _nc_cache["nc"]


def make_in_maps(query, key_, value, Wq, bq, Wk, bk, Wv, bv, wg, bg, Wo, bo, Wd, bd, Wh, bh):
    """Host-side sharding: returns (in_maps for 8 cores, fused bias)."""
    f = np.asarray
    Wf = f(Wo, np.float64) @ f(Wd, np.float64) @ f(Wh, np.float64)
    bf = (f(bo, np.float64) @ f(Wd, np.float64) @ f(Wh, np.float64)
          + f(bd, np.float64) @ f(Wh, np.float64) + f(bh, np.float64))

    wg4 = np.zeros((C, HPC), np.float32)
    for h in range(HPC):
        wg4[h * DH:(h + 1) * DH, h] = np.asarray(wg, np.float32)
    wg4 = wg4.astype(BF16)
    bg128 = np.full((128, 1), np.float32(bg), np.float32)

    xT = []
    for b in range(B):
        xT.append(tuple(
            np.ascontiguousarray(np.asarray(x[b], np.float32).T).astype(BF16)
            for x in (query, key_, value)
        ))

    in_maps = []
    for c in range(NCORES):
        b, g = divmod(c, HPC)
        cols = slice(g * C, (g + 1) * C)
        qTb, kTb, vTb = xT[b]
        in_maps.append({
            "qT": qTb, "kT": kTb, "vT": vTb,
            "wq": np.ascontiguousarray(np.asarray(Wq, np.float32)[:, cols]).astype(BF16),
            "wk": np.ascontiguousarray(np.asarray(Wk, np.float32)[:, cols]).astype(BF16),
            "wv": np.ascontiguousarray(np.asarray(Wv, np.float32)[:, cols]).astype(BF16),
            "wf": np.ascontiguousarray(Wf[cols, :]).astype(BF16),
            "wg4": wg4, "bg128": bg128,
            "bq": np.asarray(bq, np.float32)[None, cols].astype(BF16),
            "bk": np.asarray(bk, np.float32)[None, cols].astype(BF16),
            "bv": np.asarray(bv, np.float32)[None, cols].astype(BF16),
        })
    return in_maps, bf.astype(np.float32)


def gather(results, bf):
    out = np.zeros((B, S, D), np.float32)
    for c in range(NCORES):
        b = c // HPC
        out[b] += np.asarray(results[c]["out"], np.float32)
    out += bf[None, None, :]
    return out


def kernel(**inputs):
    from concourse.bass_utils import run_bass_kernel_spmd

    nc = get_nc()
    in_maps, bf = make_in_maps(**inputs)
    res = run_bass_kernel_spmd(nc, in_maps, core_ids=list(range(NCORES)))
    return gather(res.results, bf)

